# revision 46
# baseline (speedup 1.0000x reference)
"""Trainium2 Bass kernel for the CNN-MAD per-class DTW transport cost.

Math (reference):
  mat_cost[n, j] = C1[n] + C2[c_n, j] - 2*C3[n, j],  c_n = classes[n]
    C1[n]    = sum_t rowsum[c_n, t] * ||X[n,t,:]||^2
    C2[c, j] = sum_p colsum[c, p] * ||Y[j,p,:]||^2
    C3[n, j] = sum_{p,d} (sum_t pi[c_n,t,p] X[n,t,d]) * Y[j,p,d]

Sharding: one class per core (C == n_cores == 8). Host groups samples by
class (pure gather / re-layout, no arithmetic), each core computes the
[NY, CAP] transposed block for its class against the full Y, and the host
scatters rows back into the [N, NY] output.

Device per core (class k), all f32:
  - pi    [T, TP]      : class-k DTW matrix
  - xt2   [T, D*CAP]   : X.T re-layout, xt2[t, d*CAP+n] = Xg[n, t, d]
  - yt    [D*TP, NY]   : Y.T re-layout, yt[d*TP+p, j]  = Y[j, p, d]
  rowsum via DVE reduce; colsum via matmul with ones; C1 via matmul of
  rowsum over squared xt2; C2 via ACT squares and a fused DVE
  scale-accumulate chain plus a ones-contraction matmul; XW = pi.T @ X
  per d; final transposed result outT[j, n] accumulated kc-outer across
  8 concurrently-open PSUM banks as sum_kc yt_kc.T @ (-2*XW)_kc with a
  trailing [C2;1].T [1;C1] augmentation matmul per 128-row block.
"""

import sys

sys.path.insert(0, "/opt/trn_rl_repo")

import numpy as np

N, NY, T, TP, D, C = 1024, 1024, 256, 256, 8, 8
NCORES = 8

_cache = {}


def _build(cap):
    import concourse.bacc as bacc
    import concourse.mybir as mybir
    import concourse.tile as tile

    dt = mybir.dt.float32
    nc = bacc.Bacc("TRN2", target_bir_lowering=False, debug=False, num_devices=NCORES)

    pi_d = nc.dram_tensor("pi", [T, TP], dt, kind="ExternalInput")
    xt2_d = nc.dram_tensor("xt2", [T, D * cap], dt, kind="ExternalInput")
    yt_d = nc.dram_tensor("yt", [D * TP, NY], dt, kind="ExternalInput")
    out_d = nc.dram_tensor("outT", [NY, cap], dt, kind="ExternalOutput")

    KC = D * TP // 128  # 16 yt chunks of 128 contraction rows
    XF = D * cap        # xt2 free size
    JT = NY // 128      # 8 output partition tiles (transposed layout)
    # XW psum segments, aligned to d-blocks and <= 512 f32 (one PSUM bank)
    DSEG = [(0, 3), (3, 3), (6, 2)]

    with tile.TileContext(nc) as tc:
        with (
            tc.tile_pool(name="const", bufs=1) as pconst,
            tc.tile_pool(name="xin", bufs=1) as px,
            tc.tile_pool(name="ytp", bufs=1) as pyt,
            tc.tile_pool(name="ysqw", bufs=6) as pysq,
            tc.tile_pool(name="xwt", bufs=1) as pxwt,
            tc.tile_pool(name="osb", bufs=8) as posb,
            tc.tile_pool(name="ps", bufs=8, space="PSUM") as psp,
        ):
            # ---- input DMAs: pi/xt2 on gpsimd SWDGE, yt chunks on SP HWDGE ----
            pi_sb = []
            for tch in range(2):
                p = pconst.tile([128, TP], dt, tag=f"pi{tch}")
                nc.sync.dma_start(p[:], pi_d[tch * 128 : (tch + 1) * 128, :])
                pi_sb.append(p)
            xt2 = []
            for tch in range(2):
                xt = px.tile([128, XF], dt, tag=f"xt2_{tch}")
                for d0, nd in [(0, 3), (3, 3), (6, 2)]:
                    nc.sync.dma_start(
                        xt[:, d0 * cap : (d0 + nd) * cap],
                        xt2_d[
                            tch * 128 : (tch + 1) * 128,
                            d0 * cap : (d0 + nd) * cap,
                        ],
                    )
                xt2.append(xt)
            yt = pyt.tile([128, KC * NY], dt, tag="yt")
            for kc in range(KC):
                nc.sync.dma_start(
                    yt[:, kc * NY : (kc + 1) * NY],
                    yt_d[kc * 128 : (kc + 1) * 128, :],
                )

            # ---- rowsum (DVE free-dim reduce), ones, colsum (PE) ----
            rowsum = []
            for tch in range(2):
                r = pconst.tile([128, 1], dt, tag=f"rowsum{tch}")
                nc.vector.reduce_sum(r[:], pi_sb[tch][:], axis=mybir.AxisListType.X)
                rowsum.append(r)
            ones = pconst.tile([128, 1], dt, tag="ones")
            nc.vector.memset(ones[:], 1.0)

            # ---- XW: per p-half, out [128p, (d,n)] = pi_half.T @ xt2 ----
            # xwt viewed [128, d, pc, n]: chunk kc = d*2+pc of (-2*XW).T
            xwt = pxwt.tile([128, KC * cap], dt, tag="xwt")
            xwt_v = xwt.rearrange("l (d pc n) -> l d pc n", pc=2, n=cap)
            xw_ps = {
                (pc, d0): psp.tile(
                    [128, nd * cap], dt, tag="ps8", name=f"xwps{pc}_{d0}"
                )
                for pc in range(2)
                for d0, nd in DSEG
            }
            # tch-outer so PE starts on xt2[0] before xt2[1] lands
            for tch in range(2):
                for pc in range(2):
                    for d0, nd in DSEG:
                        nc.tensor.matmul(
                            xw_ps[(pc, d0)][:],
                            pi_sb[tch][:, pc * 128 : (pc + 1) * 128],
                            xt2[tch][:, d0 * cap : (d0 + nd) * cap],
                            start=(tch == 0),
                            stop=(tch == 1),
                        )
            for pc in range(2):
                for d0, nd in DSEG:
                    # ACT evac with -2 scale into strided chunk layout
                    nc.scalar.mul(
                        xwt_v[:, d0 : d0 + nd, pc, :],
                        xw_ps[(pc, d0)].rearrange("l (d n) -> l d n", n=cap),
                        -2.0,
                    )

            # ---- colsum (PE, after XW so XW leads the PE stream) ----
            cs_ps = psp.tile([128, 2], dt, tag="ps8", name="cs_ps")
            for pc in range(2):
                for tch in range(2):
                    nc.tensor.matmul(
                        cs_ps[:, pc : pc + 1],
                        pi_sb[tch][:, pc * 128 : (pc + 1) * 128],
                        ones[:],
                        start=(tch == 0),
                        stop=(tch == 1),
                    )
            colsum_sb = pconst.tile([128, 2], dt, tag="colsum_sb")
            nc.vector.tensor_copy(colsum_sb[:], cs_ps[:])
            colsum = [colsum_sb[:, 0:1], colsum_sb[:, 1:2]]

            # ---- xt2 squares + C1 row ----
            xt2sq = []
            for tch in range(2):
                xsq = px.tile([128, XF], dt, tag=f"xt2sq_{tch}")
                nc.scalar.square(xsq[:], xt2[tch][:])
                xt2sq.append(xsq)

            # ---- C2 partial sums: squares split ACT/Pool + fused DVE
            # scale-accumulate (after the XW evacs so ACT frees xwt first) ----
            ssum = pconst.tile([128, NY], dt, tag="ssum")
            for kc in range(KC):
                ysq = pysq.tile([128, NY], dt, tag="ysq")
                ysrc = yt[:, kc * NY : (kc + 1) * NY]
                if kc < 5:
                    # Pool is free early; ACT is busy with XW evacs at first
                    nc.gpsimd.tensor_mul(ysq[:], ysrc, ysrc)
                else:
                    nc.scalar.square(ysq[:], ysrc)
                if kc == 0:
                    nc.vector.tensor_scalar_mul(ssum[:], ysq[:], colsum[0][:])
                else:
                    nc.vector.scalar_tensor_tensor(
                        ssum[:],
                        ysq[:],
                        colsum[kc % 2][:],
                        ssum[:],
                        op0=mybir.AluOpType.mult,
                        op1=mybir.AluOpType.add,
                    )
            # d-reduce the squares on DVE, then one small K=128 contraction
            c1row = pconst.tile([1, cap], dt, tag="c1row")
            c1_ps = psp.tile([1, cap], dt, tag="ps8", name="c1_ps")
            xsq_dsum = []
            for tch in range(2):
                xd = px.tile([128, cap], dt, tag=f"xsq_dsum{tch}")
                nc.vector.reduce_sum(
                    xd[:],
                    xt2sq[tch].rearrange("l (d n) -> l n d", n=cap),
                    axis=mybir.AxisListType.X,
                )
                xsq_dsum.append(xd)
            for tch in range(2):
                nc.tensor.matmul(
                    c1_ps[0:1, :],
                    rowsum[tch][:],
                    xsq_dsum[tch][:],
                    start=(tch == 0),
                    stop=(tch == 1),
                )
            nc.vector.tensor_copy(c1row[0:1, :], c1_ps[0:1, :])
            # aug rhs [2, cap]: row0 = ones, row1 = C1row (SBUF->SBUF DMA;
            # compute engines cannot write at partition base 1)
            aug_r = pconst.tile([2, cap], dt, tag="aug_r")
            nc.vector.memset(aug_r[:], 1.0)
            nc.sync.dma_start(aug_r[1:2, :], c1row[0:1, :])

            # ---- C2 row: ones-contraction of ssum (own 2-bank pool, so slot
            # waits never block the C3 PE stream) ----
            aug_l = pconst.tile([2, NY], dt, tag="aug_l")
            nc.vector.memset(aug_l[:], 1.0)
            # partition-axis reduction on Pool, straight into aug_l row 0
            nc.gpsimd.reduce_sum(
                aug_l[0:1, :], ssum[:], axis=mybir.AxisListType.C
            )

            # ---- C3 transposed, kc-outer, all 8 groups open at once: three
            # jt-groups share each PSUM bank (cap*3 <= 512 f32) ----
            gsz = 512 // cap  # groups per psum tile
            ntile = -(-JT // gsz)
            pstiles = [
                psp.tile([128, min(gsz, JT - i * gsz) * cap], dt, tag="ps8",
                         name=f"psc3_{i}")
                for i in range(ntile)
            ]

            def pslice(jt):
                return pstiles[jt // gsz][:, (jt % gsz) * cap : (jt % gsz + 1) * cap]

            for kc in range(KC):
                for jt in range(JT):
                    nc.tensor.matmul(
                        pslice(jt),
                        yt[:, kc * NY + jt * 128 : kc * NY + (jt + 1) * 128],
                        xwt[:, kc * cap : (kc + 1) * cap],
                        start=(kc == 0 and jt % gsz == 0),
                        stop=False,
                        skip_group_check=True,
                    )
            # close groups bank-major: all augs of a bank, then its evacs, so
            # the same-bank PE-write/DVE-read serialization never ping-pongs
            for i in range(ntile):
                jts = range(i * gsz, min((i + 1) * gsz, JT))
                for jt in jts:
                    nc.tensor.matmul(
                        pslice(jt),
                        aug_l[:, jt * 128 : (jt + 1) * 128],
                        aug_r[:],
                        start=False,
                        stop=True,
                    )
                osb = posb.tile(
                    [128, len(jts) * cap], dt, tag=f"osb{i}", name=f"osb{i}"
                )
                for k, jt in enumerate(jts):
                    nc.vector.tensor_copy(
                        osb[:, k * cap : (k + 1) * cap], pslice(jt)
                    )
                # one DMA per bank: DRAM view [l, jt, n] pairs with SBUF
                # [l(part), jt, n]
                j0 = i * gsz
                nc.sync.dma_start(
                    out_d.rearrange("(jt l) n -> l jt n", l=128)[
                        :, j0 : j0 + len(jts), :
                    ],
                    osb.rearrange("l (jt n) -> l jt n", n=cap),
                )

    nc.compile()
    return nc


def kernel(X, Y, pi_dtw, classes):
    from concourse.bass_utils import run_bass_kernel_spmd

    X = np.ascontiguousarray(np.asarray(X, dtype=np.float32))
    Y = np.ascontiguousarray(np.asarray(Y, dtype=np.float32))
    pi_dtw = np.ascontiguousarray(np.asarray(pi_dtw, dtype=np.float32))
    classes = np.asarray(classes).astype(np.int64)

    counts = np.bincount(classes, minlength=C)
    cap = max(96, int(-(-int(counts.max()) // 8) * 8))

    if cap not in _cache:
        _cache[cap] = _build(cap)
    nc = _cache[cap]

    # host-side re-layouts (data movement only, no arithmetic)
    yt = np.ascontiguousarray(Y.transpose(2, 1, 0).reshape(D * TP, NY))
    idx = [np.nonzero(classes == c)[0] for c in range(C)]
    in_maps = []
    for c in range(C):
        xg = np.zeros((cap, T, D), dtype=np.float32)
        xg[: counts[c]] = X[idx[c]]
        xt2 = np.ascontiguousarray(xg.transpose(1, 2, 0).reshape(T, D * cap))
        in_maps.append(
            {"pi": np.ascontiguousarray(pi_dtw[c]), "xt2": xt2, "yt": yt}
        )

    res = run_bass_kernel_spmd(nc, in_maps, core_ids=list(range(NCORES)))

    out = np.empty((N, NY), dtype=np.float32)
    for c in range(C):
        out[idx[c]] = res.results[c]["outT"].T[: counts[c]]
    return out


# revision 67
# speedup vs baseline: 1.2158x; 1.2158x over previous
"""Trainium2 Bass kernel for the CNN-MAD per-class DTW transport cost.

Math (reference):
  mat_cost[n, j] = C1[n] + C2[c_n, j] - 2*C3[n, j],  c_n = classes[n]
    C1[n]    = sum_t rowsum[c_n, t] * ||X[n,t,:]||^2
    C2[c, j] = sum_p colsum[c, p] * ||Y[j,p,:]||^2
    C3[n, j] = sum_{p,d} (sum_t pi[c_n,t,p] X[n,t,d]) * Y[j,p,d]

Sharding: one class per core (C == n_cores == 8). Host groups samples by
class (pure gather / re-layout, no arithmetic), each core computes the
[NY, CAP] transposed block for its class against the full Y, and the host
scatters rows back into the [N, NY] output.

Device per core (class k), all f32:
  - pi    [T, TP]      : class-k DTW matrix
  - xt2   [T, D*CAP]   : X.T re-layout, xt2[t, d*CAP+n] = Xg[n, t, d]
  - yt    [D*TP, NY]   : Y.T re-layout, yt[d*TP+p, j]  = Y[j, p, d]
  rowsum via DVE reduce; colsum via matmul with ones; C1 via matmul of
  rowsum over squared xt2; C2 via ACT squares and a fused DVE
  scale-accumulate chain plus a ones-contraction matmul; XW = pi.T @ X
  per d; final transposed result outT[j, n] accumulated kc-outer across
  8 concurrently-open PSUM banks as sum_kc yt_kc.T @ (-2*XW)_kc with a
  trailing [C2;1].T [1;C1] augmentation matmul per 128-row block.
"""

import sys

sys.path.insert(0, "/opt/trn_rl_repo")

import numpy as np

N, NY, T, TP, D, C = 1024, 1024, 256, 256, 8, 8
NCORES = 8

_cache = {}


def _build(cap):
    import concourse.bacc as bacc
    import concourse.mybir as mybir
    import concourse.tile as tile

    dt = mybir.dt.float32
    nc = bacc.Bacc("TRN2", target_bir_lowering=False, debug=False, num_devices=NCORES)

    pi_d = nc.dram_tensor("pi", [T, TP], dt, kind="ExternalInput")
    xt2_d = nc.dram_tensor("xt2", [T, D * cap], dt, kind="ExternalInput")
    yt_d = nc.dram_tensor("yt", [D * TP, NY], dt, kind="ExternalInput")
    out_d = nc.dram_tensor("outT", [NY, cap], dt, kind="ExternalOutput")

    KC = D * TP // 128  # 16 yt chunks of 128 contraction rows
    XF = D * cap        # xt2 free size
    JT = NY // 128      # 8 output partition tiles (transposed layout)
    # XW psum segments, aligned to d-blocks and <= 512 f32 (one PSUM bank)
    DSEG = [(0, 3), (3, 3), (6, 2)]

    with tile.TileContext(nc) as tc:
        with (
            tc.tile_pool(name="const", bufs=1) as pconst,
            tc.tile_pool(name="xin", bufs=1) as px,
            tc.tile_pool(name="ytp", bufs=1) as pyt,
            tc.tile_pool(name="ysqw", bufs=6) as pysq,
            tc.tile_pool(name="xwt", bufs=1) as pxwt,
            tc.tile_pool(name="osb", bufs=8) as posb,
            tc.tile_pool(name="ps", bufs=8, space="PSUM") as psp,
        ):
            # ---- input DMAs: pi/xt2 on gpsimd SWDGE, yt chunks on SP HWDGE ----
            pi_sb = []
            for tch in range(2):
                p = pconst.tile([128, TP], dt, tag=f"pi{tch}")
                nc.sync.dma_start(p[:], pi_d[tch * 128 : (tch + 1) * 128, :])
                pi_sb.append(p)
            xt2 = []
            for tch in range(2):
                xt = px.tile([128, XF], dt, tag=f"xt2_{tch}")
                for d0, nd in [(0, 3), (3, 3), (6, 2)]:
                    nc.sync.dma_start(
                        xt[:, d0 * cap : (d0 + nd) * cap],
                        xt2_d[
                            tch * 128 : (tch + 1) * 128,
                            d0 * cap : (d0 + nd) * cap,
                        ],
                    )
                xt2.append(xt)
            yt = pyt.tile([128, KC * NY], dt, tag="yt")
            for kc in range(KC):
                nc.sync.dma_start(
                    yt[:, kc * NY : (kc + 1) * NY],
                    yt_d[kc * 128 : (kc + 1) * 128, :],
                )

            # ---- rowsum (DVE free-dim reduce), ones, colsum (PE) ----
            rowsum = []
            for tch in range(2):
                r = pconst.tile([128, 1], dt, tag=f"rowsum{tch}")
                nc.vector.reduce_sum(r[:], pi_sb[tch][:], axis=mybir.AxisListType.X)
                rowsum.append(r)
            ones = pconst.tile([128, 1], dt, tag="ones")
            nc.vector.memset(ones[:], 1.0)

            cs_ps = psp.tile([128, 2], dt, tag="ps8", name="cs_ps")
            for pc in range(2):
                for tch in range(2):
                    nc.tensor.matmul(
                        cs_ps[:, pc : pc + 1],
                        pi_sb[tch][:, pc * 128 : (pc + 1) * 128],
                        ones[:],
                        start=(tch == 0),
                        stop=(tch == 1),
                    )
            colsum_sb = pconst.tile([128, 2], dt, tag="colsum_sb")
            nc.vector.tensor_copy(colsum_sb[:], cs_ps[:])
            colsum = [colsum_sb[:, 0:1], colsum_sb[:, 1:2]]

            # ---- XW: per p-half, out [128p, (d,n)] = pi_half.T @ xt2 ----
            # xwt viewed [128, d, pc, n]: chunk kc = d*2+pc of (-2*XW).T
            xwt = pxwt.tile([128, KC * cap], dt, tag="xwt")
            xwt_v = xwt.rearrange("l (d pc n) -> l d pc n", pc=2, n=cap)
            xw_ps = {
                (pc, d0): psp.tile(
                    [128, nd * cap], dt, tag="ps8", name=f"xwps{pc}_{d0}"
                )
                for pc in range(2)
                for d0, nd in DSEG
            }
            # tch-outer so PE starts on xt2[0] before xt2[1] lands
            for tch in range(2):
                for pc in range(2):
                    for d0, nd in DSEG:
                        nc.tensor.matmul(
                            xw_ps[(pc, d0)][:],
                            pi_sb[tch][:, pc * 128 : (pc + 1) * 128],
                            xt2[tch][:, d0 * cap : (d0 + nd) * cap],
                            start=(tch == 0),
                            stop=(tch == 1),
                        )
            for pc in range(2):
                for d0, nd in DSEG:
                    # ACT evac with -2 scale into strided chunk layout
                    nc.scalar.mul(
                        xwt_v[:, d0 : d0 + nd, pc, :],
                        xw_ps[(pc, d0)].rearrange("l (d n) -> l d n", n=cap),
                        -2.0,
                    )

            # ---- xt2 squares + C1 row ----
            xt2sq = []
            for tch in range(2):
                xsq = px.tile([128, XF], dt, tag=f"xt2sq_{tch}")
                nc.scalar.square(xsq[:], xt2[tch][:])
                xt2sq.append(xsq)

            # ---- C2 partial sums: squares split ACT/Pool + fused DVE
            # scale-accumulate (after the XW evacs so ACT frees xwt first) ----
            ssum = pconst.tile([128, NY], dt, tag="ssum")
            for kc in range(KC):
                ysq = pysq.tile([128, NY], dt, tag="ysq")
                ysrc = yt[:, kc * NY : (kc + 1) * NY]
                if kc < 5:
                    # Pool is free early; ACT is busy with XW evacs at first
                    nc.gpsimd.tensor_mul(ysq[:], ysrc, ysrc)
                else:
                    nc.scalar.square(ysq[:], ysrc)
                if kc == 0:
                    nc.vector.tensor_scalar_mul(ssum[:], ysq[:], colsum[0][:])
                else:
                    nc.vector.scalar_tensor_tensor(
                        ssum[:],
                        ysq[:],
                        colsum[kc % 2][:],
                        ssum[:],
                        op0=mybir.AluOpType.mult,
                        op1=mybir.AluOpType.add,
                    )
            # d-reduce the squares on DVE, then one small K=128 contraction
            c1row = pconst.tile([1, cap], dt, tag="c1row")
            c1_ps = psp.tile([1, cap], dt, tag="ps8", name="c1_ps")
            xsq_dsum = []
            for tch in range(2):
                xd = px.tile([128, cap], dt, tag=f"xsq_dsum{tch}")
                nc.vector.reduce_sum(
                    xd[:],
                    xt2sq[tch].rearrange("l (d n) -> l n d", n=cap),
                    axis=mybir.AxisListType.X,
                )
                xsq_dsum.append(xd)
            for tch in range(2):
                nc.tensor.matmul(
                    c1_ps[0:1, :],
                    rowsum[tch][:],
                    xsq_dsum[tch][:],
                    start=(tch == 0),
                    stop=(tch == 1),
                )
            nc.vector.tensor_copy(c1row[0:1, :], c1_ps[0:1, :])
            # aug rhs [2, cap]: row0 = ones, row1 = C1row (SBUF->SBUF DMA;
            # compute engines cannot write at partition base 1)
            aug_r = pconst.tile([2, cap], dt, tag="aug_r")
            nc.vector.memset(aug_r[:], 1.0)
            nc.sync.dma_start(aug_r[1:2, :], c1row[0:1, :])

            # ---- C2 row: ones-contraction of ssum (own 2-bank pool, so slot
            # waits never block the C3 PE stream) ----
            aug_l = pconst.tile([2, NY], dt, tag="aug_l")
            nc.vector.memset(aug_l[:], 1.0)
            # partition-axis reduction on Pool, straight into aug_l row 0
            nc.gpsimd.reduce_sum(
                aug_l[0:1, :], ssum[:], axis=mybir.AxisListType.C
            )

            # ---- C3 transposed, kc-outer, all 8 groups open at once: three
            # jt-groups share each PSUM bank (cap*3 <= 512 f32) ----
            gsz = 512 // cap  # groups per psum tile
            ntile = -(-JT // gsz)
            pstiles = [
                psp.tile([128, min(gsz, JT - i * gsz) * cap], dt, tag="ps8",
                         name=f"psc3_{i}")
                for i in range(ntile)
            ]

            def pslice(jt):
                return pstiles[jt // gsz][:, (jt % gsz) * cap : (jt % gsz + 1) * cap]

            for kc in range(KC):
                for jt in range(JT):
                    nc.tensor.matmul(
                        pslice(jt),
                        yt[:, kc * NY + jt * 128 : kc * NY + (jt + 1) * 128],
                        xwt[:, kc * cap : (kc + 1) * cap],
                        start=(kc == 0 and jt % gsz == 0),
                        stop=False,
                        skip_group_check=True,
                    )
            # close groups bank-major: all augs of a bank, then its evacs, so
            # the same-bank PE-write/DVE-read serialization never ping-pongs
            for i in range(ntile):
                jts = range(i * gsz, min((i + 1) * gsz, JT))
                for jt in jts:
                    nc.tensor.matmul(
                        pslice(jt),
                        aug_l[:, jt * 128 : (jt + 1) * 128],
                        aug_r[:],
                        start=False,
                        stop=True,
                    )
                osb = posb.tile(
                    [128, len(jts) * cap], dt, tag=f"osb{i}", name=f"osb{i}"
                )
                for k, jt in enumerate(jts):
                    nc.vector.tensor_copy(
                        osb[:, k * cap : (k + 1) * cap], pslice(jt)
                    )
                # one DMA per bank: DRAM view [l, jt, n] pairs with SBUF
                # [l(part), jt, n]
                j0 = i * gsz
                nc.sync.dma_start(
                    out_d.rearrange("(jt l) n -> l jt n", l=128)[
                        :, j0 : j0 + len(jts), :
                    ],
                    osb.rearrange("l (jt n) -> l jt n", n=cap),
                )

    nc.compile()
    return nc


def kernel(X, Y, pi_dtw, classes):
    from concourse.bass_utils import run_bass_kernel_spmd

    X = np.ascontiguousarray(np.asarray(X, dtype=np.float32))
    Y = np.ascontiguousarray(np.asarray(Y, dtype=np.float32))
    pi_dtw = np.ascontiguousarray(np.asarray(pi_dtw, dtype=np.float32))
    classes = np.asarray(classes).astype(np.int64)

    counts = np.bincount(classes, minlength=C)
    cap = max(96, int(-(-int(counts.max()) // 8) * 8))

    if cap not in _cache:
        _cache[cap] = _build(cap)
    nc = _cache[cap]

    # host-side re-layouts (data movement only, no arithmetic)
    yt = np.ascontiguousarray(Y.transpose(2, 1, 0).reshape(D * TP, NY))
    idx = [np.nonzero(classes == c)[0] for c in range(C)]
    in_maps = []
    for c in range(C):
        xg = np.zeros((cap, T, D), dtype=np.float32)
        xg[: counts[c]] = X[idx[c]]
        xt2 = np.ascontiguousarray(xg.transpose(1, 2, 0).reshape(T, D * cap))
        in_maps.append(
            {"pi": np.ascontiguousarray(pi_dtw[c]), "xt2": xt2, "yt": yt}
        )

    res = run_bass_kernel_spmd(nc, in_maps, core_ids=list(range(NCORES)))

    out = np.empty((N, NY), dtype=np.float32)
    for c in range(C):
        out[idx[c]] = res.results[c]["outT"].T[: counts[c]]
    return out
